# revision 7
# baseline (speedup 1.0000x reference)
"""ConvLoRA fused kernel for Trainium2 (8 NeuronCores, data-parallel over batch).

Math: conv is linear in its weight, so
    org + outA + outB = conv(x[b], conv_w + wA[b] + wB[b]) + conv_b
One fused per-sample 3x3 conv (256->256ch) in bf16. Per-sample weights are
generated on-device: since both LoRA branches share lora_B,
    wA + wB = B @ (C1@A1 + C2@A2)
so the two branches accumulate into one PSUM "AS" [16, 768] which is then
expanded tap-by-tap via strided matmuls against a zero-padded copy (ASE) --
out-of-segment reads land in zeros, handling the p%3 block boundaries exactly.

Conv streams contiguous 3-row windows (N=388) from a whole-image SBUF buffer
with 130-elem row pitch (halo cols are zero); the 2 junk columns per row
boundary are discarded at PSUM evacuation. One matmul per stationary weight;
bf16 fast-weight-load hides under the 388-cycle matmul.
"""
import sys
sys.path.insert(0, '/opt/trn_rl_repo')
import numpy as np
import ml_dtypes

import concourse.bacc as bacc
import concourse.mybir as mybir
import concourse.tile as tile
from concourse.bass_utils import run_bass_kernel_spmd

f32 = mybir.dt.float32
f32r = mybir.dt.float32r
bf16 = mybir.dt.bfloat16
AF = mybir.ActivationFunctionType

B, CIN, COUT, KS, H, W, R = 16, 256, 256, 3, 128, 128, 16
NCORES = 8
NB = B // NCORES   # 2 samples per core
PITCH = W + 2      # 130: one zero col each side
NTILE = 43         # ceil(128/3): 42 full 3-row tiles + one 2-row tile


def _build_nc():
    nc = bacc.Bacc("TRN2", target_bir_lowering=False, debug=False, num_devices=NCORES)

    x_loc = nc.dram_tensor("x_loc", [NB, 2, 128, H + 2, PITCH], bf16, kind="ExternalInput")
    wm = nc.dram_tensor("wm", [33, 2 * NB], f32, kind="ExternalInput")
    ew1 = nc.dram_tensor("ew1", [33, 256], f32, kind="ExternalInput")
    w2t = nc.dram_tensor("w2t", [128, 512], f32, kind="ExternalInput")
    b2x = nc.dram_tensor("b2x", [16, 2 * 16 * NB], f32, kind="ExternalInput")
    loraA = nc.dram_tensor("loraA", [32, 768], f32, kind="ExternalInput")
    lba = nc.dram_tensor("lba", [16, 3, 256], f32r, kind="ExternalInput")
    wbase = nc.dram_tensor("wbase", [128, 9, 2, 256], bf16, kind="ExternalInput")
    convb = nc.dram_tensor("convb", [128, 2], f32, kind="ExternalInput")
    out = nc.dram_tensor("out", [NB, COUT, H, W], f32, kind="ExternalOutput")

    with tile.TileContext(nc) as tc:
        from contextlib import ExitStack
        with ExitStack() as ctx:
            cpools = ctx.enter_context(tc.tile_pool(name="consts", bufs=1))
            w18pool = ctx.enter_context(tc.tile_pool(name="w18", bufs=9 * 2 * NB))
            sb_wg = ctx.enter_context(tc.tile_pool(name="sbwg", bufs=1))
            ps_w = ctx.enter_context(tc.tile_pool(name="psw", bufs=1, space="PSUM"))
            xpool = ctx.enter_context(tc.tile_pool(name="ximg", bufs=2 * NB))
            stg = ctx.enter_context(tc.tile_pool(name="stg", bufs=4))

            # zero-padded AS expansion buffers (per sample), zeroed off the
            # critical path on gpsimd
            ASE = []
            for bi in range(NB):
                a = sb_wg.tile([16, 2304], f32r, tag="ase", bufs=NB)
                nc.gpsimd.memset(a[:].bitcast(f32), 0.0)
                ASE.append(a)

            # ---- constants (small / MLP-critical first) ----
            wm_sb = cpools.tile([33, 2 * NB], f32)
            nc.sync.dma_start(wm_sb[:], wm[:])
            ew1_sb = cpools.tile([33, 256], f32)
            nc.sync.dma_start(ew1_sb[:], ew1[:])
            w2t_sb = cpools.tile([128, 512], f32)
            nc.sync.dma_start(w2t_sb[:], w2t[:])
            b2x_sb = cpools.tile([16, 2 * 16 * NB], f32)
            nc.sync.dma_start(b2x_sb[:], b2x[:])
            lba_sb = cpools.tile([16, 3, 256], f32r)
            nc.sync.dma_start(lba_sb[:], lba[:])
            convb_sb = cpools.tile([128, 2], f32)
            nc.sync.dma_start(convb_sb[:], convb[:])
            wbase_sb = cpools.tile([128, 9, 2, 256], bf16)
            nc.sync.dma_start(wbase_sb[:, 0:2], wbase[:, 0:2])

            # ---- x image buffers (bi=0 now; bi=1 later) ----
            xf = [[None] * 2 for _ in range(NB)]

            def load_x(bi):
                # host-padded 130x130 image: fully-contiguous DMA in row bands
                bands = [(0, 18)] + [(18 + 16 * b, 34 + 16 * b) for b in range(7)]
                for j in range(2):
                    xx = xpool.tile([128, PITCH * (H + 2)], bf16, tag="ximg")
                    xr = xx[:].rearrange("p (a b) -> p a b", b=PITCH)
                    for r0, r1 in bands:
                        nc.sync.dma_start(xr[:, r0:r1, :], x_loc[bi, j, :, r0:r1, :])
                    xf[bi][j] = xx

            load_x(0)
            nc.sync.dma_start(wbase_sb[:, 2:9], wbase[:, 2:9])

            # ---- MLP (shared) + AS for both samples ----
            coff_sb = []
            with ExitStack() as actx:
                mlp_sb = actx.enter_context(tc.tile_pool(name="mlpw", bufs=1))
                ps_h = actx.enter_context(tc.tile_pool(name="psh", bufs=1, space="PSUM"))
                ps_c = actx.enter_context(tc.tile_pool(name="psc", bufs=2, space="PSUM"))
                ps_a = actx.enter_context(tc.tile_pool(name="psa", bufs=1, space="PSUM"))

                loraA1_sb = mlp_sb.tile([16, 768], f32, tag="la", bufs=2)
                loraA2_sb = mlp_sb.tile([16, 768], f32, tag="la", bufs=2)
                loraA_sb = [loraA1_sb, loraA2_sb]
                nc.sync.dma_start(loraA_sb[0][:], loraA[0:16, :])
                nc.sync.dma_start(loraA_sb[1][:], loraA[16:32, :])

                haug = mlp_sb.tile([128, 2 * NB], f32)
                for br in range(2):
                    h_ps = ps_h.tile([128, NB], f32, tag="hps")
                    nc.tensor.matmul(h_ps[:], ew1_sb[:, 128 * br:128 * (br + 1)],
                                     wm_sb[:, NB * br:NB * (br + 1)], start=True, stop=True)
                    # leaky relu slope 0.2 == max(0.2*x, x)
                    h_sb = mlp_sb.tile([128, NB], f32, tag="hsb", bufs=2)
                    nc.scalar.activation(h_sb[:], h_ps[:], AF.Copy)
                    nc.vector.scalar_tensor_tensor(
                        haug[:, NB * br:NB * (br + 1)], h_sb[:], 0.2, h_sb[:],
                        mybir.AluOpType.mult, mybir.AluOpType.max)
                # stage 2 -> coff[br] (16, 16, NB) [q, r, bi]
                b2x_r = b2x_sb[:, :].rearrange("q (br r b) -> q br r b", br=2, b=NB)
                for br in range(2):
                    c_ps = ps_c.tile([16, 16, NB], f32, tag="cps")
                    for r in range(16):
                        nc.tensor.matmul(c_ps[:, r, :],
                                         w2t_sb[:, 256 * br + 16 * r:256 * br + 16 * (r + 1)],
                                         haug[:, NB * br:NB * (br + 1)],
                                         start=True, stop=True)
                    csb = cpools.tile([16, 16, NB], f32, tag="coff", bufs=2)
                    nc.vector.tensor_add(csb[:], c_ps[:], b2x_r[:, br])
                    coff_sb.append(csb)
                # AS[bi] = coff1[bi].T @ A1 + coff2[bi].T @ A2  -> ASE center
                for bi in range(NB):
                    a_ps = ps_a.tile([16, 768], f32, tag="aps")
                    for c0, c1 in ((0, 512), (512, 768)):
                        nc.tensor.matmul(a_ps[:, c0:c1], coff_sb[0][:, :, bi],
                                         loraA_sb[0][:, c0:c1], start=True, stop=False)
                        nc.tensor.matmul(a_ps[:, c0:c1], coff_sb[1][:, :, bi],
                                         loraA_sb[1][:, c0:c1], start=False, stop=True)
                    nc.vector.tensor_copy(ASE[bi][:, 768:1536], a_ps[:])

            w18 = [[[None] * 2 for _ in range(9)] for _ in range(NB)]

            def emit_wgen(bi):
                # W18[bi][t][j] = (AS expansion) @ lba + wbase, in bf16
                ase_r = ASE[bi][:].rearrange("p (c n) -> p c n", n=9)
                for t in range(9):
                    for j in range(2):
                        wg = ps_w.tile([128, 256], f32, tag="wg")
                        for idx, a in enumerate((j, j + 1)):
                            base = 768 + 1152 * j + t - 768 * a
                            c0, n0 = divmod(base, 9)
                            nc.tensor.matmul(wg[:], ase_r[:, c0:c0 + 128, n0],
                                             lba_sb[:, a, :],
                                             start=(idx == 0), stop=(idx == 1))
                        wt = w18pool.tile([128, 256], bf16, tag="w18")
                        nc.vector.tensor_add(wt[:], wg[:], wbase_sb[:, t, j])
                        w18[bi][t][j] = wt

            emit_wgen(0)

            # ---- the conv ----
            with ExitStack() as bctx:
                cps = bctx.enter_context(tc.tile_pool(name="cps", bufs=6, space="PSUM"))

                def conv_pass(bi, oc):
                    for k in range(NTILE):
                        nr = 3 if k < NTILE - 1 else 2
                        n = PITCH * (nr - 1) + W
                        ps = cps.tile([128, 512], f32, tag="cps")
                        w = 0
                        for kh in range(3):
                            for kw in range(3):
                                t = 3 * kh + kw
                                off = PITCH * (3 * k + kh) + kw
                                for j in range(2):
                                    nc.tensor.matmul(
                                        ps[:, 0:n],
                                        w18[bi][t][j][:, 128 * oc:128 * (oc + 1)],
                                        xf[bi][j][:, off:off + n],
                                        start=(w == 0), stop=(w == 17))
                                    w += 1
                        st = stg.tile([128, 3, 128], f32, tag="stg")
                        pv = ps[:, 0:PITCH * nr].rearrange("p (a b) -> p a b", b=PITCH)
                        nc.vector.tensor_scalar_add(st[:, 0:nr, :], pv[:, :, 0:W],
                                                    convb_sb[:, oc:oc + 1])
                        nc.sync.dma_start(
                            out[bi, 128 * oc:128 * (oc + 1), 3 * k:3 * k + nr, :],
                            st[:, 0:nr, :])

                conv_pass(0, 0)
                emit_wgen(1)
                load_x(1)
                conv_pass(0, 1)
                conv_pass(1, 0)
                conv_pass(1, 1)
    nc.finalize()
    return nc


def _host_prep(inputs):
    """Prepare replicated / per-core numpy input maps."""
    x = np.asarray(inputs["x"], dtype=np.float32)
    wms = np.asarray(inputs["wms"], dtype=np.float32)
    conv_w = np.asarray(inputs["conv_w"], dtype=np.float32)
    conv_b = np.asarray(inputs["conv_b"], dtype=np.float32)
    e_w1 = [np.asarray(inputs["e1_w1"], np.float32), np.asarray(inputs["e2_w1"], np.float32)]
    e_b1 = [np.asarray(inputs["e1_b1"], np.float32), np.asarray(inputs["e2_b1"], np.float32)]
    e_w2 = [np.asarray(inputs["e1_w2"], np.float32), np.asarray(inputs["e2_w2"], np.float32)]
    e_b2 = [np.asarray(inputs["e1_b2"], np.float32), np.asarray(inputs["e2_b2"], np.float32)]
    lora_A = [np.asarray(inputs["lora_A1"], np.float32), np.asarray(inputs["lora_A2"], np.float32)]
    lora_B = np.asarray(inputs["lora_B"], np.float32)

    ew1 = np.zeros((33, 256), np.float32)
    for br in range(2):
        ew1[:32, 128 * br:128 * (br + 1)] = e_w1[br].T
        ew1[32, 128 * br:128 * (br + 1)] = e_b1[br]
    w2t = np.concatenate([e_w2[0].T, e_w2[1].T], axis=1).astype(np.float32)
    b2x = np.zeros((16, 2, 16, NB), np.float32)
    for br in range(2):
        b2x[:, br, :, :] = e_b2[br].reshape(16, 16).T[:, :, None]
    b2x = np.ascontiguousarray(b2x.reshape(16, 2 * 16 * NB))
    loraA = np.concatenate([lora_A[0], lora_A[1]], 0).astype(np.float32)
    # lba[r, a, cout] = lora_B[3*cout + a, r]
    lba = np.ascontiguousarray(lora_B.reshape(256, 3, 16).transpose(2, 1, 0))
    # wbase[p, t, j, cout] = conv_w[cout, 128j+p, t//3, t%3]
    wbase = np.ascontiguousarray(
        conv_w.transpose(2, 3, 1, 0).reshape(9, 2, 128, 256).transpose(2, 0, 1, 3)
    ).astype(ml_dtypes.bfloat16)
    convb = np.ascontiguousarray(conv_b.reshape(2, 128).T)

    xp = np.zeros((B, 2, 128, H + 2, W + 2), dtype=ml_dtypes.bfloat16)
    xp[:, :, :, 1:H + 1, 1:W + 1] = x.reshape(B, 2, 128, H, W).astype(ml_dtypes.bfloat16)
    in_maps = []
    for core in range(NCORES):
        b0 = core * NB
        wmc = np.ones((33, 2 * NB), np.float32)
        for br in range(2):
            for bi in range(NB):
                wmc[:32, NB * br + bi] = wms[br, b0 + bi]
        in_maps.append({
            "x_loc": np.ascontiguousarray(xp[b0:b0 + NB]),
            "wm": wmc, "ew1": ew1, "w2t": w2t, "b2x": b2x,
            "loraA": loraA, "lba": lba, "wbase": wbase, "convb": convb,
        })
    return in_maps


_NC = None


def kernel(**inputs) -> np.ndarray:
    global _NC
    if _NC is None:
        _NC = _build_nc()
    in_maps = _host_prep(inputs)
    res = run_bass_kernel_spmd(_NC, in_maps, core_ids=list(range(NCORES)))
    return np.concatenate([res.results[c]["out"] for c in range(NCORES)], axis=0)


# revision 10
# speedup vs baseline: 1.1076x; 1.1076x over previous
"""ConvLoRA fused kernel for Trainium2 (8 NeuronCores, data-parallel over batch).

Math: conv is linear in its weight, so
    org + outA + outB = conv(x[b], conv_w + wA[b] + wB[b]) + conv_b
One fused per-sample 3x3 conv (256->256ch) in bf16. Per-sample weights are
generated on-device: since both LoRA branches share lora_B,
    wA + wB = B @ (C1@A1 + C2@A2)
so the two branches accumulate into one PSUM "AS" [16, 768] which is then
expanded tap-by-tap via strided matmuls against a zero-padded copy (ASE) --
out-of-segment reads land in zeros, handling the p%3 block boundaries exactly.

Conv streams contiguous 3-row windows (N=388) from a whole-image SBUF buffer
with 130-elem row pitch (halo cols are zero); the 2 junk columns per row
boundary are discarded at PSUM evacuation. One matmul per stationary weight;
bf16 fast-weight-load hides under the 388-cycle matmul.
"""
import sys
sys.path.insert(0, '/opt/trn_rl_repo')
import numpy as np
import ml_dtypes

import concourse.bacc as bacc
import concourse.mybir as mybir
import concourse.tile as tile
from concourse.bass_utils import run_bass_kernel_spmd

f32 = mybir.dt.float32
f32r = mybir.dt.float32r
bf16 = mybir.dt.bfloat16
AF = mybir.ActivationFunctionType

B, CIN, COUT, KS, H, W, R = 16, 256, 256, 3, 128, 128, 16
NCORES = 8
NB = B // NCORES   # 2 samples per core
PITCH = W + 2      # 130: one zero col each side
NTILE = 43         # ceil(128/3): 42 full 3-row tiles + one 2-row tile


def _build_nc():
    nc = bacc.Bacc("TRN2", target_bir_lowering=False, debug=False, num_devices=NCORES)

    x_loc = nc.dram_tensor("x_loc", [NB, 2, 128, H + 2, PITCH], bf16, kind="ExternalInput")
    wm = nc.dram_tensor("wm", [33, 2 * NB], f32, kind="ExternalInput")
    ew1 = nc.dram_tensor("ew1", [33, 256], f32, kind="ExternalInput")
    w2t = nc.dram_tensor("w2t", [128, 512], f32, kind="ExternalInput")
    b2x = nc.dram_tensor("b2x", [16, 2 * 16 * NB], f32, kind="ExternalInput")
    loraA = nc.dram_tensor("loraA", [32, 768], f32, kind="ExternalInput")
    lba = nc.dram_tensor("lba", [16, 3, 256], f32r, kind="ExternalInput")
    wbase = nc.dram_tensor("wbase", [128, 9, 2, 256], bf16, kind="ExternalInput")
    convb = nc.dram_tensor("convb", [128, 2], f32, kind="ExternalInput")
    out = nc.dram_tensor("out", [NB, COUT, H, W], f32, kind="ExternalOutput")

    with tile.TileContext(nc) as tc:
        from contextlib import ExitStack
        with ExitStack() as ctx:
            cpools = ctx.enter_context(tc.tile_pool(name="consts", bufs=1))
            w18pool = ctx.enter_context(tc.tile_pool(name="w18", bufs=9 * 2 * NB))
            sb_wg = ctx.enter_context(tc.tile_pool(name="sbwg", bufs=1))
            ps_w = ctx.enter_context(tc.tile_pool(name="psw", bufs=1, space="PSUM"))
            xpool = ctx.enter_context(tc.tile_pool(name="ximg", bufs=2 * NB))
            stg = ctx.enter_context(tc.tile_pool(name="stg", bufs=4))

            # zero-padded AS expansion buffers (per sample), zeroed off the
            # critical path on gpsimd
            ASE = []
            for bi in range(NB):
                a = sb_wg.tile([16, 2304], f32r, tag="ase", bufs=NB)
                nc.gpsimd.memset(a[:].bitcast(f32), 0.0)
                ASE.append(a)

            # ---- first x rows: on the DMA critical path, issue before all
            # else, split across partition quarters for engine parallelism ----
            xf = [[None] * 2 for _ in range(NB)]
            xr0 = [None, None]
            for j in range(2):
                xx0 = xpool.tile([128, PITCH * (H + 2)], bf16, tag="ximg")
                xr0[j] = xx0[:].rearrange("p (a b) -> p a b", b=PITCH)
                xf[0][j] = xx0
            for p0 in range(0, 128, 32):
                for j in range(2):
                    nc.sync.dma_start(xr0[j][p0:p0 + 32, 0:18, :],
                                      x_loc[0, j, p0:p0 + 32, 0:18, :])

            # ---- constants (small / MLP-critical first) ----
            wm_sb = cpools.tile([33, 2 * NB], f32)
            nc.sync.dma_start(wm_sb[:], wm[:])
            ew1_sb = cpools.tile([33, 256], f32)
            nc.sync.dma_start(ew1_sb[:], ew1[:])
            w2t_sb = cpools.tile([128, 512], f32)
            nc.sync.dma_start(w2t_sb[:], w2t[:])
            b2x_sb = cpools.tile([16, 2 * 16 * NB], f32)
            nc.sync.dma_start(b2x_sb[:], b2x[:])
            lba_sb = cpools.tile([16, 3, 256], f32r)
            nc.sync.dma_start(lba_sb[:], lba[:])
            convb_sb = cpools.tile([128, 2], f32)
            nc.sync.dma_start(convb_sb[:], convb[:])
            wbase_sb = cpools.tile([128, 9, 2, 256], bf16)
            nc.sync.dma_start(wbase_sb[:, 0:2], wbase[:, 0:2])

            # ---- remaining x rows for bi=0 (chunk-interleaved row bands) ----
            for b in range(7):
                r0, r1 = 18 + 16 * b, 34 + 16 * b
                for j in range(2):
                    nc.sync.dma_start(xr0[j][:, r0:r1, :], x_loc[0, j, :, r0:r1, :])
            nc.sync.dma_start(wbase_sb[:, 2:9], wbase[:, 2:9])

            def load_x(bi):
                # host-padded 130x130 image: fully-contiguous DMA in row bands
                bands = [(0, 18)] + [(18 + 16 * b, 34 + 16 * b) for b in range(7)]
                for j in range(2):
                    xx = xpool.tile([128, PITCH * (H + 2)], bf16, tag="ximg")
                    xr = xx[:].rearrange("p (a b) -> p a b", b=PITCH)
                    for r0, r1 in bands:
                        nc.sync.dma_start(xr[:, r0:r1, :], x_loc[bi, j, :, r0:r1, :])
                    xf[bi][j] = xx

            # ---- MLP (shared) + AS for both samples ----
            coff_sb = []
            with ExitStack() as actx:
                mlp_sb = actx.enter_context(tc.tile_pool(name="mlpw", bufs=1))
                ps_h = actx.enter_context(tc.tile_pool(name="psh", bufs=1, space="PSUM"))
                ps_c = actx.enter_context(tc.tile_pool(name="psc", bufs=2, space="PSUM"))
                ps_a = actx.enter_context(tc.tile_pool(name="psa", bufs=1, space="PSUM"))

                loraA1_sb = mlp_sb.tile([16, 768], f32, tag="la", bufs=2)
                loraA2_sb = mlp_sb.tile([16, 768], f32, tag="la", bufs=2)
                loraA_sb = [loraA1_sb, loraA2_sb]
                nc.sync.dma_start(loraA_sb[0][:], loraA[0:16, :])
                nc.sync.dma_start(loraA_sb[1][:], loraA[16:32, :])

                haug = mlp_sb.tile([128, 2 * NB], f32)
                for br in range(2):
                    h_ps = ps_h.tile([128, NB], f32, tag="hps")
                    nc.tensor.matmul(h_ps[:], ew1_sb[:, 128 * br:128 * (br + 1)],
                                     wm_sb[:, NB * br:NB * (br + 1)], start=True, stop=True)
                    # leaky relu slope 0.2 == max(0.2*x, x)
                    h_sb = mlp_sb.tile([128, NB], f32, tag="hsb", bufs=2)
                    nc.scalar.activation(h_sb[:], h_ps[:], AF.Copy)
                    nc.vector.scalar_tensor_tensor(
                        haug[:, NB * br:NB * (br + 1)], h_sb[:], 0.2, h_sb[:],
                        mybir.AluOpType.mult, mybir.AluOpType.max)
                # stage 2 -> coff[br] (16, 16, NB) [q, r, bi]
                b2x_r = b2x_sb[:, :].rearrange("q (br r b) -> q br r b", br=2, b=NB)
                for br in range(2):
                    c_ps = ps_c.tile([16, 16, NB], f32, tag="cps")
                    for r in range(16):
                        nc.tensor.matmul(c_ps[:, r, :],
                                         w2t_sb[:, 256 * br + 16 * r:256 * br + 16 * (r + 1)],
                                         haug[:, NB * br:NB * (br + 1)],
                                         start=True, stop=True)
                    csb = cpools.tile([16, 16, NB], f32, tag="coff", bufs=2)
                    nc.vector.tensor_add(csb[:], c_ps[:], b2x_r[:, br])
                    coff_sb.append(csb)
                # AS[bi] = coff1[bi].T @ A1 + coff2[bi].T @ A2  -> ASE center
                for bi in range(NB):
                    a_ps = ps_a.tile([16, 768], f32, tag="aps")
                    for c0, c1 in ((0, 512), (512, 768)):
                        nc.tensor.matmul(a_ps[:, c0:c1], coff_sb[0][:, :, bi],
                                         loraA_sb[0][:, c0:c1], start=True, stop=False)
                        nc.tensor.matmul(a_ps[:, c0:c1], coff_sb[1][:, :, bi],
                                         loraA_sb[1][:, c0:c1], start=False, stop=True)
                    nc.vector.tensor_copy(ASE[bi][:, 768:1536], a_ps[:])

            w18 = [[[None] * 2 for _ in range(9)] for _ in range(NB)]

            def emit_wgen(bi):
                # W18[bi][t][j] = (AS expansion) @ lba + wbase, in bf16
                ase_r = ASE[bi][:].rearrange("p (c n) -> p c n", n=9)
                for t in range(9):
                    for j in range(2):
                        wg = ps_w.tile([128, 256], f32, tag="wg")
                        for idx, a in enumerate((j, j + 1)):
                            base = 768 + 1152 * j + t - 768 * a
                            c0, n0 = divmod(base, 9)
                            nc.tensor.matmul(wg[:], ase_r[:, c0:c0 + 128, n0],
                                             lba_sb[:, a, :],
                                             start=(idx == 0), stop=(idx == 1))
                        wt = w18pool.tile([128, 256], bf16, tag="w18")
                        nc.vector.tensor_add(wt[:], wg[:], wbase_sb[:, t, j])
                        w18[bi][t][j] = wt

            emit_wgen(0)

            # ---- the conv ----
            with ExitStack() as bctx:
                cps = bctx.enter_context(tc.tile_pool(name="cps", bufs=6, space="PSUM"))

                def conv_pass(bi, oc):
                    for k in range(NTILE):
                        nr = 3 if k < NTILE - 1 else 2
                        n = PITCH * (nr - 1) + W
                        ps = cps.tile([128, 512], f32, tag="cps")
                        w = 0
                        for j in range(2):
                            for kh in range(3):
                                for kw in range(3):
                                    t = 3 * kh + kw
                                    off = PITCH * (3 * k + kh) + kw
                                    nc.tensor.matmul(
                                        ps[:, 0:n],
                                        w18[bi][t][j][:, 128 * oc:128 * (oc + 1)],
                                        xf[bi][j][:, off:off + n],
                                        start=(w == 0), stop=(w == 17))
                                    w += 1
                        st = stg.tile([128, 3, 128], f32, tag="stg")
                        pv = ps[:, 0:PITCH * nr].rearrange("p (a b) -> p a b", b=PITCH)
                        nc.vector.tensor_scalar_add(st[:, 0:nr, :], pv[:, :, 0:W],
                                                    convb_sb[:, oc:oc + 1])
                        nc.sync.dma_start(
                            out[bi, 128 * oc:128 * (oc + 1), 3 * k:3 * k + nr, :],
                            st[:, 0:nr, :])

                conv_pass(0, 0)
                emit_wgen(1)
                load_x(1)
                conv_pass(0, 1)
                conv_pass(1, 0)
                conv_pass(1, 1)
    nc.finalize()
    return nc


def _host_prep(inputs):
    """Prepare replicated / per-core numpy input maps."""
    x = np.asarray(inputs["x"], dtype=np.float32)
    wms = np.asarray(inputs["wms"], dtype=np.float32)
    conv_w = np.asarray(inputs["conv_w"], dtype=np.float32)
    conv_b = np.asarray(inputs["conv_b"], dtype=np.float32)
    e_w1 = [np.asarray(inputs["e1_w1"], np.float32), np.asarray(inputs["e2_w1"], np.float32)]
    e_b1 = [np.asarray(inputs["e1_b1"], np.float32), np.asarray(inputs["e2_b1"], np.float32)]
    e_w2 = [np.asarray(inputs["e1_w2"], np.float32), np.asarray(inputs["e2_w2"], np.float32)]
    e_b2 = [np.asarray(inputs["e1_b2"], np.float32), np.asarray(inputs["e2_b2"], np.float32)]
    lora_A = [np.asarray(inputs["lora_A1"], np.float32), np.asarray(inputs["lora_A2"], np.float32)]
    lora_B = np.asarray(inputs["lora_B"], np.float32)

    ew1 = np.zeros((33, 256), np.float32)
    for br in range(2):
        ew1[:32, 128 * br:128 * (br + 1)] = e_w1[br].T
        ew1[32, 128 * br:128 * (br + 1)] = e_b1[br]
    w2t = np.concatenate([e_w2[0].T, e_w2[1].T], axis=1).astype(np.float32)
    b2x = np.zeros((16, 2, 16, NB), np.float32)
    for br in range(2):
        b2x[:, br, :, :] = e_b2[br].reshape(16, 16).T[:, :, None]
    b2x = np.ascontiguousarray(b2x.reshape(16, 2 * 16 * NB))
    loraA = np.concatenate([lora_A[0], lora_A[1]], 0).astype(np.float32)
    # lba[r, a, cout] = lora_B[3*cout + a, r]
    lba = np.ascontiguousarray(lora_B.reshape(256, 3, 16).transpose(2, 1, 0))
    # wbase[p, t, j, cout] = conv_w[cout, 128j+p, t//3, t%3]
    wbase = np.ascontiguousarray(
        conv_w.transpose(2, 3, 1, 0).reshape(9, 2, 128, 256).transpose(2, 0, 1, 3)
    ).astype(ml_dtypes.bfloat16)
    convb = np.ascontiguousarray(conv_b.reshape(2, 128).T)

    xp = np.zeros((B, 2, 128, H + 2, W + 2), dtype=ml_dtypes.bfloat16)
    xp[:, :, :, 1:H + 1, 1:W + 1] = x.reshape(B, 2, 128, H, W).astype(ml_dtypes.bfloat16)
    in_maps = []
    for core in range(NCORES):
        b0 = core * NB
        wmc = np.ones((33, 2 * NB), np.float32)
        for br in range(2):
            for bi in range(NB):
                wmc[:32, NB * br + bi] = wms[br, b0 + bi]
        in_maps.append({
            "x_loc": np.ascontiguousarray(xp[b0:b0 + NB]),
            "wm": wmc, "ew1": ew1, "w2t": w2t, "b2x": b2x,
            "loraA": loraA, "lba": lba, "wbase": wbase, "convb": convb,
        })
    return in_maps


_NC = None


def kernel(**inputs) -> np.ndarray:
    global _NC
    if _NC is None:
        _NC = _build_nc()
    in_maps = _host_prep(inputs)
    res = run_bass_kernel_spmd(_NC, in_maps, core_ids=list(range(NCORES)))
    return np.concatenate([res.results[c]["out"] for c in range(NCORES)], axis=0)


# revision 12
# speedup vs baseline: 1.2278x; 1.1085x over previous
"""ConvLoRA fused kernel for Trainium2 (8 NeuronCores, data-parallel over batch).

Math: conv is linear in its weight, so
    org + outA + outB = conv(x[b], conv_w + wA[b] + wB[b]) + conv_b
One fused per-sample 3x3 conv (256->256ch) in bf16. Per-sample weights are
generated on-device: since both LoRA branches share lora_B,
    wA + wB = B @ (C1@A1 + C2@A2)
so the two branches accumulate into one PSUM "AS" [16, 768] which is then
expanded tap-by-tap via strided matmuls against a zero-padded copy (ASE) --
out-of-segment reads land in zeros, handling the p%3 block boundaries exactly.

Conv streams contiguous 3-row windows (N=388) from a whole-image SBUF buffer
with 130-elem row pitch (halo cols are zero); the 2 junk columns per row
boundary are discarded at PSUM evacuation. One matmul per stationary weight;
bf16 fast-weight-load hides under the 388-cycle matmul.
"""
import sys
sys.path.insert(0, '/opt/trn_rl_repo')
import numpy as np
import ml_dtypes

import concourse.bacc as bacc
import concourse.mybir as mybir
import concourse.tile as tile
from concourse.bass_utils import run_bass_kernel_spmd

f32 = mybir.dt.float32
f32r = mybir.dt.float32r
bf16 = mybir.dt.bfloat16
AF = mybir.ActivationFunctionType

B, CIN, COUT, KS, H, W, R = 16, 256, 256, 3, 128, 128, 16
NCORES = 8
NB = B // NCORES   # 2 samples per core
PITCH = W + 2      # 130: one zero col each side
NTILE = 43         # ceil(128/3): 42 full 3-row tiles + one 2-row tile


def _build_nc():
    nc = bacc.Bacc("TRN2", target_bir_lowering=False, debug=False, num_devices=NCORES)

    x_loc = nc.dram_tensor("x_loc", [NB, 2, 128, H + 2, PITCH], bf16, kind="ExternalInput")
    wm = nc.dram_tensor("wm", [33, 2 * NB], f32, kind="ExternalInput")
    ew1 = nc.dram_tensor("ew1", [33, 256], f32, kind="ExternalInput")
    w2t = nc.dram_tensor("w2t", [128, 512], f32, kind="ExternalInput")
    b2x = nc.dram_tensor("b2x", [16, 2 * 16 * NB], f32, kind="ExternalInput")
    loraA = nc.dram_tensor("loraA", [32, 768], f32, kind="ExternalInput")
    lba = nc.dram_tensor("lba", [16, 3, 256], f32r, kind="ExternalInput")
    wbase = nc.dram_tensor("wbase", [128, 9, 2, 256], bf16, kind="ExternalInput")
    convb = nc.dram_tensor("convb", [128, 2], f32, kind="ExternalInput")
    out = nc.dram_tensor("out", [NB, COUT, H, W], f32, kind="ExternalOutput")

    with tile.TileContext(nc) as tc:
        from contextlib import ExitStack
        with ExitStack() as ctx:
            cpools = ctx.enter_context(tc.tile_pool(name="consts", bufs=1))
            w18pool = ctx.enter_context(tc.tile_pool(name="w18", bufs=9 * 2 * NB))
            sb_wg = ctx.enter_context(tc.tile_pool(name="sbwg", bufs=1))
            ps_w = ctx.enter_context(tc.tile_pool(name="psw", bufs=1, space="PSUM"))
            xpool = ctx.enter_context(tc.tile_pool(name="ximg", bufs=2 * NB))
            stg = ctx.enter_context(tc.tile_pool(name="stg", bufs=4))

            # zero-padded AS expansion buffers (per sample), zeroed off the
            # critical path on gpsimd
            ASE = []
            for bi in range(NB):
                a = sb_wg.tile([16, 2304], f32r, tag="ase", bufs=NB)
                nc.gpsimd.memset(a[:].bitcast(f32), 0.0)
                ASE.append(a)

            # ---- MLP-critical consts first (DMA issue order IS dependency
            # order: shared semaphore rings aggregate completions) ----
            wm_sb = cpools.tile([33, 2 * NB], f32)
            nc.sync.dma_start(wm_sb[:], wm[:])
            ew1_sb = cpools.tile([33, 256], f32)
            nc.sync.dma_start(ew1_sb[:], ew1[:])

            # first 18 x rows, split across partition quarters for parallelism
            xf = [[None] * 2 for _ in range(NB)]
            xr0 = [None, None]
            for j in range(2):
                xx0 = xpool.tile([128, PITCH * (H + 2)], bf16, tag="ximg")
                xr0[j] = xx0[:].rearrange("p (a b) -> p a b", b=PITCH)
                xf[0][j] = xx0
            for p0 in range(0, 128, 32):
                for j in range(2):
                    nc.sync.dma_start(xr0[j][p0:p0 + 32, 0:18, :],
                                      x_loc[0, j, p0:p0 + 32, 0:18, :])

            # everything wgen needs, before the bulk x bands
            w2t_sb = cpools.tile([128, 512], f32)
            nc.sync.dma_start(w2t_sb[:], w2t[:])
            b2x_sb = cpools.tile([16, 2 * 16 * NB], f32)
            nc.sync.dma_start(b2x_sb[:], b2x[:])
            lba_sb = cpools.tile([16, 3, 256], f32r)
            nc.sync.dma_start(lba_sb[:], lba[:])
            convb_sb = cpools.tile([128, 2], f32)
            nc.sync.dma_start(convb_sb[:], convb[:])
            wbase_sb = cpools.tile([128, 9, 2, 256], bf16)
            nc.sync.dma_start(wbase_sb[:, 0:2], wbase[:, 0:2])
            nc.sync.dma_start(wbase_sb[:, 2:9], wbase[:, 2:9])

            def load_x(bi):
                # host-padded 130x130 image: fully-contiguous DMA in row bands
                bands = [(0, 18)] + [(18 + 16 * b, 34 + 16 * b) for b in range(7)]
                for j in range(2):
                    xx = xpool.tile([128, PITCH * (H + 2)], bf16, tag="ximg")
                    xr = xx[:].rearrange("p (a b) -> p a b", b=PITCH)
                    for r0, r1 in bands:
                        nc.sync.dma_start(xr[:, r0:r1, :], x_loc[bi, j, :, r0:r1, :])
                    xf[bi][j] = xx

            # ---- MLP (shared) + AS for both samples ----
            coff_sb = []
            with ExitStack() as actx:
                mlp_sb = actx.enter_context(tc.tile_pool(name="mlpw", bufs=1))
                ps_h = actx.enter_context(tc.tile_pool(name="psh", bufs=1, space="PSUM"))
                ps_c = actx.enter_context(tc.tile_pool(name="psc", bufs=2, space="PSUM"))
                ps_a = actx.enter_context(tc.tile_pool(name="psa", bufs=1, space="PSUM"))

                loraA1_sb = mlp_sb.tile([16, 768], f32, tag="la", bufs=2)
                loraA2_sb = mlp_sb.tile([16, 768], f32, tag="la", bufs=2)
                loraA_sb = [loraA1_sb, loraA2_sb]
                nc.sync.dma_start(loraA_sb[0][:], loraA[0:16, :])
                nc.sync.dma_start(loraA_sb[1][:], loraA[16:32, :])

                haug = mlp_sb.tile([128, 2 * NB], f32)
                for br in range(2):
                    h_ps = ps_h.tile([128, NB], f32, tag="hps")
                    nc.tensor.matmul(h_ps[:], ew1_sb[:, 128 * br:128 * (br + 1)],
                                     wm_sb[:, NB * br:NB * (br + 1)], start=True, stop=True)
                    # leaky relu slope 0.2 == max(0.2*x, x)
                    h_sb = mlp_sb.tile([128, NB], f32, tag="hsb", bufs=2)
                    nc.scalar.activation(h_sb[:], h_ps[:], AF.Copy)
                    nc.vector.scalar_tensor_tensor(
                        haug[:, NB * br:NB * (br + 1)], h_sb[:], 0.2, h_sb[:],
                        mybir.AluOpType.mult, mybir.AluOpType.max)
                # stage 2 -> coff[br] (16, 16, NB) [q, r, bi]
                b2x_r = b2x_sb[:, :].rearrange("q (br r b) -> q br r b", br=2, b=NB)
                for br in range(2):
                    c_ps = ps_c.tile([16, 16, NB], f32, tag="cps")
                    for r in range(16):
                        nc.tensor.matmul(c_ps[:, r, :],
                                         w2t_sb[:, 256 * br + 16 * r:256 * br + 16 * (r + 1)],
                                         haug[:, NB * br:NB * (br + 1)],
                                         start=True, stop=True)
                    csb = cpools.tile([16, 16, NB], f32, tag="coff", bufs=2)
                    nc.vector.tensor_add(csb[:], c_ps[:], b2x_r[:, br])
                    coff_sb.append(csb)
                # AS[bi] = coff1[bi].T @ A1 + coff2[bi].T @ A2  -> ASE center
                for bi in range(NB):
                    a_ps = ps_a.tile([16, 768], f32, tag="aps")
                    for c0, c1 in ((0, 512), (512, 768)):
                        nc.tensor.matmul(a_ps[:, c0:c1], coff_sb[0][:, :, bi],
                                         loraA_sb[0][:, c0:c1], start=True, stop=False)
                        nc.tensor.matmul(a_ps[:, c0:c1], coff_sb[1][:, :, bi],
                                         loraA_sb[1][:, c0:c1], start=False, stop=True)
                    nc.vector.tensor_copy(ASE[bi][:, 768:1536], a_ps[:])

            w18 = [[[None] * 2 for _ in range(9)] for _ in range(NB)]

            def emit_wgen(bi):
                # W18[bi][t][j] = (AS expansion) @ lba + wbase, in bf16
                ase_r = ASE[bi][:].rearrange("p (c n) -> p c n", n=9)
                for t in range(9):
                    for j in range(2):
                        wg = ps_w.tile([128, 256], f32, tag="wg")
                        for idx, a in enumerate((j, j + 1)):
                            base = 768 + 1152 * j + t - 768 * a
                            c0, n0 = divmod(base, 9)
                            nc.tensor.matmul(wg[:], ase_r[:, c0:c0 + 128, n0],
                                             lba_sb[:, a, :],
                                             start=(idx == 0), stop=(idx == 1))
                        wt = w18pool.tile([128, 256], bf16, tag="w18")
                        nc.vector.tensor_add(wt[:], wg[:], wbase_sb[:, t, j])
                        w18[bi][t][j] = wt

            emit_wgen(0)

            # remaining x rows for bi=0 (chunk-interleaved row bands), after
            # every wgen input so they don't gate the weight chain
            for b in range(7):
                r0, r1 = 18 + 16 * b, 34 + 16 * b
                for j in range(2):
                    nc.sync.dma_start(xr0[j][:, r0:r1, :], x_loc[0, j, :, r0:r1, :])

            # ---- the conv ----
            with ExitStack() as bctx:
                cps = bctx.enter_context(tc.tile_pool(name="cps", bufs=6, space="PSUM"))

                def conv_pass(bi, oc):
                    for k in range(NTILE):
                        nr = 3 if k < NTILE - 1 else 2
                        n = PITCH * (nr - 1) + W
                        ps = cps.tile([128, 512], f32, tag="cps")
                        w = 0
                        for j in range(2):
                            for kh in range(3):
                                for kw in range(3):
                                    t = 3 * kh + kw
                                    off = PITCH * (3 * k + kh) + kw
                                    nc.tensor.matmul(
                                        ps[:, 0:n],
                                        w18[bi][t][j][:, 128 * oc:128 * (oc + 1)],
                                        xf[bi][j][:, off:off + n],
                                        start=(w == 0), stop=(w == 17))
                                    w += 1
                        st = stg.tile([128, 3, 128], f32, tag="stg")
                        pv = ps[:, 0:PITCH * nr].rearrange("p (a b) -> p a b", b=PITCH)
                        nc.vector.tensor_scalar_add(st[:, 0:nr, :], pv[:, :, 0:W],
                                                    convb_sb[:, oc:oc + 1])
                        nc.sync.dma_start(
                            out[bi, 128 * oc:128 * (oc + 1), 3 * k:3 * k + nr, :],
                            st[:, 0:nr, :])

                conv_pass(0, 0)
                emit_wgen(1)
                load_x(1)
                conv_pass(0, 1)
                conv_pass(1, 0)
                conv_pass(1, 1)
    nc.finalize()
    return nc


def _host_prep(inputs):
    """Prepare replicated / per-core numpy input maps."""
    x = np.asarray(inputs["x"], dtype=np.float32)
    wms = np.asarray(inputs["wms"], dtype=np.float32)
    conv_w = np.asarray(inputs["conv_w"], dtype=np.float32)
    conv_b = np.asarray(inputs["conv_b"], dtype=np.float32)
    e_w1 = [np.asarray(inputs["e1_w1"], np.float32), np.asarray(inputs["e2_w1"], np.float32)]
    e_b1 = [np.asarray(inputs["e1_b1"], np.float32), np.asarray(inputs["e2_b1"], np.float32)]
    e_w2 = [np.asarray(inputs["e1_w2"], np.float32), np.asarray(inputs["e2_w2"], np.float32)]
    e_b2 = [np.asarray(inputs["e1_b2"], np.float32), np.asarray(inputs["e2_b2"], np.float32)]
    lora_A = [np.asarray(inputs["lora_A1"], np.float32), np.asarray(inputs["lora_A2"], np.float32)]
    lora_B = np.asarray(inputs["lora_B"], np.float32)

    ew1 = np.zeros((33, 256), np.float32)
    for br in range(2):
        ew1[:32, 128 * br:128 * (br + 1)] = e_w1[br].T
        ew1[32, 128 * br:128 * (br + 1)] = e_b1[br]
    w2t = np.concatenate([e_w2[0].T, e_w2[1].T], axis=1).astype(np.float32)
    b2x = np.zeros((16, 2, 16, NB), np.float32)
    for br in range(2):
        b2x[:, br, :, :] = e_b2[br].reshape(16, 16).T[:, :, None]
    b2x = np.ascontiguousarray(b2x.reshape(16, 2 * 16 * NB))
    loraA = np.concatenate([lora_A[0], lora_A[1]], 0).astype(np.float32)
    # lba[r, a, cout] = lora_B[3*cout + a, r]
    lba = np.ascontiguousarray(lora_B.reshape(256, 3, 16).transpose(2, 1, 0))
    # wbase[p, t, j, cout] = conv_w[cout, 128j+p, t//3, t%3]
    wbase = np.ascontiguousarray(
        conv_w.transpose(2, 3, 1, 0).reshape(9, 2, 128, 256).transpose(2, 0, 1, 3)
    ).astype(ml_dtypes.bfloat16)
    convb = np.ascontiguousarray(conv_b.reshape(2, 128).T)

    xp = np.zeros((B, 2, 128, H + 2, W + 2), dtype=ml_dtypes.bfloat16)
    xp[:, :, :, 1:H + 1, 1:W + 1] = x.reshape(B, 2, 128, H, W).astype(ml_dtypes.bfloat16)
    in_maps = []
    for core in range(NCORES):
        b0 = core * NB
        wmc = np.ones((33, 2 * NB), np.float32)
        for br in range(2):
            for bi in range(NB):
                wmc[:32, NB * br + bi] = wms[br, b0 + bi]
        in_maps.append({
            "x_loc": np.ascontiguousarray(xp[b0:b0 + NB]),
            "wm": wmc, "ew1": ew1, "w2t": w2t, "b2x": b2x,
            "loraA": loraA, "lba": lba, "wbase": wbase, "convb": convb,
        })
    return in_maps


_NC = None


def kernel(**inputs) -> np.ndarray:
    global _NC
    if _NC is None:
        _NC = _build_nc()
    in_maps = _host_prep(inputs)
    res = run_bass_kernel_spmd(_NC, in_maps, core_ids=list(range(NCORES)))
    return np.concatenate([res.results[c]["out"] for c in range(NCORES)], axis=0)


# revision 13
# speedup vs baseline: 1.2949x; 1.0546x over previous
"""ConvLoRA fused kernel, v3: 1D Winograd F(2,3) along W on the fused conv.

out[y, 2t+d] = A^T_d ( sum_{kh,cin} (G W)_i [kh] * V_i[y+kh-1, t] ),
V_i = B^T-transform of x along W (4 points per 2 output cols) -> 1.5x fewer
tensor MACs than direct.  Per-sample fused weights W = conv_w + B(C1A1+C2A2)
are generated on-device already G-transformed: G is folded into lora_A on the
host (A' [16,1024]) and into conv_w (wbasep); the p%3 block-boundary issue is
handled by the zero-padded ASE' buffer (stride-12 reads, out-of-segment ->
zeros).  V is computed on gpsimd/DVE from the host-padded 130-pitch image;
inverse transform + bias on DVE/ACT during PSUM evacuation.
"""
import sys
sys.path.insert(0, '/opt/trn_rl_repo')
import numpy as np
import ml_dtypes

import concourse.bacc as bacc
import concourse.mybir as mybir
import concourse.tile as tile
from concourse.bass_utils import run_bass_kernel_spmd

f32 = mybir.dt.float32
f32r = mybir.dt.float32r
bf16 = mybir.dt.bfloat16
AF = mybir.ActivationFunctionType
OP = mybir.AluOpType

B, CIN, COUT, KS, H, W, R = 16, 256, 256, 3, 128, 128, 16
NCORES = 8
NB = B // NCORES
PITCH = W + 2        # 130
NSLAB = 4            # 32 output rows per slab
SROWS = 34           # input rows per slab (incl halo)
NTC = W // 2         # 64 winograd tile-cols


def _build_nc():
    nc = bacc.Bacc("TRN2", target_bir_lowering=False, debug=False, num_devices=NCORES)

    x_loc = nc.dram_tensor("x_loc", [NB, 2, 128, H + 2, PITCH], bf16, kind="ExternalInput")
    wm = nc.dram_tensor("wm", [33, 2 * NB], f32, kind="ExternalInput")
    ew1 = nc.dram_tensor("ew1", [33, 256], f32, kind="ExternalInput")
    w2t = nc.dram_tensor("w2t", [128, 512], f32, kind="ExternalInput")
    b2x = nc.dram_tensor("b2x", [16, 2 * 16 * NB], f32, kind="ExternalInput")
    loraAp = nc.dram_tensor("loraAp", [32, 1024], f32, kind="ExternalInput")
    lba = nc.dram_tensor("lba", [16, 3, 256], f32r, kind="ExternalInput")
    wbasep = nc.dram_tensor("wbasep", [128, 12, 2, 256], bf16, kind="ExternalInput")
    convb = nc.dram_tensor("convb", [128, 2], f32, kind="ExternalInput")
    out = nc.dram_tensor("out", [NB, COUT, H, W], f32, kind="ExternalOutput")

    with tile.TileContext(nc) as tc:
        from contextlib import ExitStack
        with ExitStack() as ctx:
            cpools = ctx.enter_context(tc.tile_pool(name="consts", bufs=1))
            w24pool = ctx.enter_context(tc.tile_pool(name="w24", bufs=12 * 2 * NB))
            sb_wg = ctx.enter_context(tc.tile_pool(name="sbwg", bufs=1))
            ps_w = ctx.enter_context(tc.tile_pool(name="psw", bufs=1, space="PSUM"))
            xpool = ctx.enter_context(tc.tile_pool(name="xslab", bufs=4))
            vpool = ctx.enter_context(tc.tile_pool(name="vslab", bufs=16))
            stg = ctx.enter_context(tc.tile_pool(name="stg", bufs=3))
            stg2 = ctx.enter_context(tc.tile_pool(name="stg2", bufs=9))

            ASE = []
            for bi in range(NB):
                a = sb_wg.tile([16, 3072], f32r, tag="ase", bufs=NB)
                nc.gpsimd.memset(a[:].bitcast(f32), 0.0)
                ASE.append(a)

            xs = [None, None]   # current x slab tiles (per chunk)
            vt = [[None, None] for _ in range(4)]  # V[i][j] current slab

            def load_slab(bi, s, split):
                r0 = 32 * s
                for j in range(2):
                    xx = xpool.tile([128, SROWS * PITCH], bf16, tag="xslab")
                    xr = xx[:].rearrange("p (a b) -> p a b", b=PITCH)
                    if split:
                        for p0 in range(0, 128, 32):
                            nc.sync.dma_start(xr[p0:p0 + 32, 0:18, :],
                                              x_loc[bi, j, p0:p0 + 32, r0:r0 + 18, :])
                        nc.sync.dma_start(xr[:, 18:SROWS, :],
                                          x_loc[bi, j, :, r0 + 18:r0 + SROWS, :])
                    else:
                        nc.sync.dma_start(xr[:, :, :], x_loc[bi, j, :, r0:r0 + SROWS, :])
                    xs[j] = xx

            def v_transform(split):
                # V0 = b0-b2, V1 = b1+b2, V2 = b2-b1, V3 = b1-b3
                for j in range(2):
                    xsr = xs[j][:].rearrange("p (a b c) -> p a b c", b=PITCH // 2, c=2)
                    b0 = xsr[:, :, 0:NTC, 0]
                    b1 = xsr[:, :, 0:NTC, 1]
                    b2 = xsr[:, :, 1:NTC + 1, 0]
                    b3 = xsr[:, :, 1:NTC + 1, 1]
                    srcs = [(b0, b2, OP.subtract), (b1, b2, OP.add),
                            (b2, b1, OP.subtract), (b1, b3, OP.subtract)]
                    for i in range(4):
                        v = vpool.tile([128, SROWS * NTC], bf16, tag="vslab")
                        vr = v[:].rearrange("p (a b) -> p a b", b=NTC)
                        eng = nc.vector if (split and i < 2) else nc.gpsimd
                        eng.tensor_tensor(vr[:, :, :], srcs[i][0], srcs[i][1], srcs[i][2])
                        vt[i][j] = v

            # ---- MLP-critical consts, then first slab x (issue order is
            # dependency order on the shared DMA semaphore rings) ----
            wm_sb = cpools.tile([33, 2 * NB], f32)
            nc.sync.dma_start(wm_sb[:], wm[:])
            ew1_sb = cpools.tile([33, 256], f32)
            nc.sync.dma_start(ew1_sb[:], ew1[:])
            load_slab(0, 0, split=True)
            w2t_sb = cpools.tile([128, 512], f32)
            nc.sync.dma_start(w2t_sb[:], w2t[:])
            b2x_sb = cpools.tile([16, 2 * 16 * NB], f32)
            nc.sync.dma_start(b2x_sb[:], b2x[:])
            lba_sb = cpools.tile([16, 3, 256], f32r)
            nc.sync.dma_start(lba_sb[:], lba[:])
            convb_sb = cpools.tile([128, 2], f32)
            nc.sync.dma_start(convb_sb[:], convb[:])
            wbasep_sb = cpools.tile([128, 12, 2, 256], bf16)
            for p0 in range(0, 128, 32):
                nc.sync.dma_start(wbasep_sb[p0:p0 + 32, 0:6], wbasep[p0:p0 + 32, 0:6])
            for p0 in range(0, 128, 32):
                nc.sync.dma_start(wbasep_sb[p0:p0 + 32, 6:12], wbasep[p0:p0 + 32, 6:12])

            # ---- MLP + AS' for both samples ----
            coff_sb = []
            with ExitStack() as actx:
                mlp_sb = actx.enter_context(tc.tile_pool(name="mlpw", bufs=1))
                ps_h = actx.enter_context(tc.tile_pool(name="psh", bufs=1, space="PSUM"))
                ps_c = actx.enter_context(tc.tile_pool(name="psc", bufs=2, space="PSUM"))
                ps_a = actx.enter_context(tc.tile_pool(name="psa", bufs=1, space="PSUM"))

                loraA1_sb = mlp_sb.tile([16, 1024], f32, tag="la", bufs=2)
                loraA2_sb = mlp_sb.tile([16, 1024], f32, tag="la", bufs=2)
                loraA_sb = [loraA1_sb, loraA2_sb]
                nc.sync.dma_start(loraA_sb[0][:], loraAp[0:16, :])
                nc.sync.dma_start(loraA_sb[1][:], loraAp[16:32, :])

                haug = mlp_sb.tile([128, 2 * NB], f32)
                for br in range(2):
                    h_ps = ps_h.tile([128, NB], f32, tag="hps")
                    nc.tensor.matmul(h_ps[:], ew1_sb[:, 128 * br:128 * (br + 1)],
                                     wm_sb[:, NB * br:NB * (br + 1)], start=True, stop=True)
                    h_sb = mlp_sb.tile([128, NB], f32, tag="hsb", bufs=2)
                    nc.scalar.activation(h_sb[:], h_ps[:], AF.Copy)
                    nc.vector.scalar_tensor_tensor(
                        haug[:, NB * br:NB * (br + 1)], h_sb[:], 0.2, h_sb[:],
                        OP.mult, OP.max)
                b2x_r = b2x_sb[:, :].rearrange("q (br r b) -> q br r b", br=2, b=NB)
                for br in range(2):
                    c_ps = ps_c.tile([16, 16, NB], f32, tag="cps2")
                    for r in range(16):
                        nc.tensor.matmul(c_ps[:, r, :],
                                         w2t_sb[:, 256 * br + 16 * r:256 * br + 16 * (r + 1)],
                                         haug[:, NB * br:NB * (br + 1)],
                                         start=True, stop=True)
                    csb = cpools.tile([16, 16, NB], f32, tag="coff", bufs=2)
                    nc.vector.tensor_add(csb[:], c_ps[:], b2x_r[:, br])
                    coff_sb.append(csb)
                for bi in range(NB):
                    a_ps = ps_a.tile([16, 1024], f32, tag="aps")
                    for c0, c1 in ((0, 512), (512, 1024)):
                        nc.tensor.matmul(a_ps[:, c0:c1], coff_sb[0][:, :, bi],
                                         loraA_sb[0][:, c0:c1], start=True, stop=False)
                        nc.tensor.matmul(a_ps[:, c0:c1], coff_sb[1][:, :, bi],
                                         loraA_sb[1][:, c0:c1], start=False, stop=True)
                    nc.vector.tensor_copy(ASE[bi][:, 1024:2048], a_ps[:])

            w24 = [[[None] * 2 for _ in range(12)] for _ in range(NB)]

            def emit_wgen(bi):
                ase_r = ASE[bi][:].rearrange("p (c n) -> p c n", n=12)
                for tp in range(12):
                    for j in range(2):
                        wg = ps_w.tile([128, 256], f32, tag="wg")
                        for idx, a in enumerate((j, j + 1)):
                            base = 1024 + 1536 * j + tp - 1024 * a
                            c0, n0 = divmod(base, 12)
                            nc.tensor.matmul(wg[:], ase_r[:, c0:c0 + 128, n0],
                                             lba_sb[:, a, :],
                                             start=(idx == 0), stop=(idx == 1))
                        wt = w24pool.tile([128, 256], bf16, tag="w24")
                        nc.vector.tensor_add(wt[:], wg[:], wbasep_sb[:, tp, j])
                        w24[bi][tp][j] = wt

            v_transform(split=True)
            emit_wgen(0)

            # ---- conv ----
            with ExitStack() as bctx:
                cps = bctx.enter_context(tc.tile_pool(name="cps", bufs=7, space="PSUM"))

                def conv_slab(bi, s, oc):
                    for k in range(4):
                        psA = []
                        for i in range(4):
                            ps = cps.tile([128, 512], f32, tag="cps")
                            w = 0
                            for kh in range(3):
                                for j in range(2):
                                    nc.tensor.matmul(
                                        ps[:],
                                        w24[bi][4 * kh + i][j][:, 128 * oc:128 * (oc + 1)],
                                        vt[i][j][:, NTC * (8 * k + kh):NTC * (8 * k + kh) + 512],
                                        start=(w == 0), stop=(w == 5))
                                    w += 1
                            psA.append(ps)
                        # inverse transform: even = M0+M1+M2+b, odd = M1-M2-M3+b
                        # (ACT stages M1/M2 to SBUF so DVE ops read <=1 PSUM)
                        a1 = stg2.tile([128, 512], f32, tag="stg2")
                        nc.scalar.activation(a1[:], psA[1][:], AF.Copy)
                        a2 = stg2.tile([128, 512], f32, tag="stg2")
                        nc.scalar.activation(a2[:], psA[2][:], AF.Copy)
                        t1 = stg2.tile([128, 512], f32, tag="stg2")
                        nc.vector.tensor_tensor(t1[:], psA[0][:], a1[:], OP.add)
                        t2 = stg2.tile([128, 512], f32, tag="stg2")
                        nc.vector.tensor_tensor(t2[:], t1[:], a2[:], OP.add)
                        t3 = stg2.tile([128, 512], f32, tag="stg2")
                        nc.vector.tensor_tensor(t3[:], a1[:], a2[:], OP.subtract)
                        t4 = stg2.tile([128, 512], f32, tag="stg2")
                        nc.vector.tensor_tensor(t4[:], t3[:], psA[3][:], OP.subtract)
                        st = stg.tile([128, 8, 128], f32, tag="stg")
                        str_ = st[:].rearrange("p a (b c) -> p a b c", c=2)
                        t2r = t2[:].rearrange("p (a b) -> p a b", b=NTC)
                        t4r = t4[:].rearrange("p (a b) -> p a b", b=NTC)
                        nc.vector.tensor_scalar_add(str_[:, :, :, 0], t2r[:],
                                                    convb_sb[:, oc:oc + 1])
                        nc.vector.tensor_scalar_add(str_[:, :, :, 1], t4r[:],
                                                    convb_sb[:, oc:oc + 1])
                        y0 = 32 * s + 8 * k
                        nc.sync.dma_start(
                            out[bi, 128 * oc:128 * (oc + 1), y0:y0 + 8, :], st[:])

                for bi in range(NB):
                    for s in range(NSLAB):
                        conv_slab(bi, s, 0)
                        if bi == 0 and s == 0:
                            emit_wgen(1)
                        if s + 1 < NSLAB:
                            load_slab(bi, s + 1, split=False)
                        elif bi + 1 < NB:
                            load_slab(bi + 1, 0, split=False)
                        conv_slab(bi, s, 1)
                        if s + 1 < NSLAB:
                            v_transform(split=False)
                        elif bi + 1 < NB:
                            v_transform(split=False)
    nc.finalize()
    return nc


G = np.array([[1, 0, 0], [.5, .5, .5], [.5, -.5, .5], [0, 0, 1]], np.float32)


def _host_prep(inputs):
    x = np.asarray(inputs["x"], dtype=np.float32)
    wms = np.asarray(inputs["wms"], dtype=np.float32)
    conv_w = np.asarray(inputs["conv_w"], dtype=np.float32)
    conv_b = np.asarray(inputs["conv_b"], dtype=np.float32)
    e_w1 = [np.asarray(inputs["e1_w1"], np.float32), np.asarray(inputs["e2_w1"], np.float32)]
    e_b1 = [np.asarray(inputs["e1_b1"], np.float32), np.asarray(inputs["e2_b1"], np.float32)]
    e_w2 = [np.asarray(inputs["e1_w2"], np.float32), np.asarray(inputs["e2_w2"], np.float32)]
    e_b2 = [np.asarray(inputs["e1_b2"], np.float32), np.asarray(inputs["e2_b2"], np.float32)]
    lora_A = [np.asarray(inputs["lora_A1"], np.float32), np.asarray(inputs["lora_A2"], np.float32)]
    lora_B = np.asarray(inputs["lora_B"], np.float32)

    ew1 = np.zeros((33, 256), np.float32)
    for br in range(2):
        ew1[:32, 128 * br:128 * (br + 1)] = e_w1[br].T
        ew1[32, 128 * br:128 * (br + 1)] = e_b1[br]
    w2t = np.concatenate([e_w2[0].T, e_w2[1].T], axis=1).astype(np.float32)
    b2x = np.zeros((16, 2, 16, NB), np.float32)
    for br in range(2):
        b2x[:, br, :, :] = e_b2[br].reshape(16, 16).T[:, :, None]
    b2x = np.ascontiguousarray(b2x.reshape(16, 2 * 16 * NB))
    # G-folded A': A'[q, 4T+i] = sum_kw G[i,kw] A[q, 3T+kw]
    def gfold(A):
        return np.einsum('ik,qtk->qti', G, A.reshape(R, 256, 3)).reshape(R, 1024)
    loraAp = np.concatenate([gfold(lora_A[0]), gfold(lora_A[1])], 0).astype(np.float32)
    lba = np.ascontiguousarray(lora_B.reshape(256, 3, 16).transpose(2, 1, 0))
    # wbasep[p, 4kh+i, j, cout] = sum_kw G[i,kw] conv_w[cout, 128j+p, kh, kw]
    wb = np.einsum('ik,ocnk->nioc', G, conv_w)           # [kh, i, cout, cin]
    wb = wb.reshape(3, 4, 256, 2, 128).transpose(4, 0, 1, 3, 2)  # [p, kh, i, j, cout]
    wbasep = np.ascontiguousarray(wb.reshape(128, 12, 2, 256)).astype(ml_dtypes.bfloat16)
    convb = np.ascontiguousarray(conv_b.reshape(2, 128).T)

    xp = np.zeros((B, 2, 128, H + 2, W + 2), dtype=ml_dtypes.bfloat16)
    xp[:, :, :, 1:H + 1, 1:W + 1] = x.reshape(B, 2, 128, H, W).astype(ml_dtypes.bfloat16)
    in_maps = []
    for core in range(NCORES):
        b0 = core * NB
        wmc = np.ones((33, 2 * NB), np.float32)
        for br in range(2):
            for bi in range(NB):
                wmc[:32, NB * br + bi] = wms[br, b0 + bi]
        in_maps.append({
            "x_loc": np.ascontiguousarray(xp[b0:b0 + NB]),
            "wm": wmc, "ew1": ew1, "w2t": w2t, "b2x": b2x,
            "loraAp": loraAp, "lba": lba, "wbasep": wbasep, "convb": convb,
        })
    return in_maps


_NC = None


def kernel(**inputs) -> np.ndarray:
    global _NC
    if _NC is None:
        _NC = _build_nc()
    in_maps = _host_prep(inputs)
    res = run_bass_kernel_spmd(_NC, in_maps, core_ids=list(range(NCORES)))
    return np.concatenate([res.results[c]["out"] for c in range(NCORES)], axis=0)
